# revision 47
# baseline (speedup 1.0000x reference)
"""BinaryLinear 2:4 kernel for trn2 (8 NeuronCores).

Computes: out = binarize(weight * mask_2_4(weight)) @ x
  - mask_2_4: keep 2 largest-|.| of every 4 along the reduction dim
  - binarize: kept positive -> 1.0, else 0.0
  - out = wb @ x, (4096, 4096) fp32

Sharding: 4 (weight rows) x 2 (x cols) grid. Core c=(mi,ni) takes weight
rows [mi*1024,(mi+1)*1024) and x cols [ni*2048,(ni+1)*2048), computing a
(1024, 2048) block; host assembles. Per-core HBM traffic 56MiB (~160us)
stays under the f16 tensor-engine floor (~220us).

Per-core schedule (mb-major wavefront): all four 512-col x chunks stay
resident in SBUF as f16 (128KiB/partition). For each 128-row weight block:
compute the 2:4 binarized mask (phase A), transpose it into a rotating
lhsT buffer, then immediately run all 4 n-chunk PSUM groups (27.6us of PE
work) against it. Mask production (~22us/block) stays ahead of PE
consumption, so the PE streams without long stalls and HAM stays warm.

Phase A per (128, 2048) chunk, all exact fp32 on DVE:
  t = 2nd-largest(|w|) per group of 4 = max(min(hi1,hi2), max(lo1,lo2))
  wb_j = (w_j >= t)  -- one contiguous op with a stride-0 broadcast of t
  (sign folded in since t > 0; verified bit-identical to jax top_k mask)
"""

import numpy as np

M_FULL = 4096
K_FULL = 4096
N_FULL = 4096
N_CORES = 8
GRID_M = 4
GRID_N = 2
M_SHARD = M_FULL // GRID_M   # 1024 weight rows per core
N_SHARD = N_FULL // GRID_N   # 2048 x cols per core

_CACHE = {}


def _build_bass(M=M_SHARD, K=K_FULL, N=N_SHARD, k_sub=1024, n_chunk=512, xp=4):
    import concourse.bass as bass
    import concourse.tile as tile
    from concourse import bacc, mybir
    from concourse.bass import broadcast_tensor_aps
    from contextlib import ExitStack

    dt = mybir.dt
    f32 = dt.float32
    f16 = dt.float16
    i32 = dt.int32
    Alu = mybir.AluOpType
    Act = mybir.ActivationFunctionType

    P = 128
    MB = M // P            # 8 m-blocks per core
    KS = K // k_sub        # 2 phase-A chunks per m-block
    KT = K // P            # 32 contraction tiles
    NCH = N // n_chunk     # 4 x chunks, all resident
    Q = k_sub // 4         # 512 groups per chunk
    TPS = k_sub // P       # 16 k-tiles per chunk

    nc = bacc.Bacc()
    w_d = nc.declare_dram_parameter("w", [M, K], f32, isOutput=False)
    x_d = nc.declare_dram_parameter("x", [K, N], f32, isOutput=False)
    out_d = nc.declare_dram_parameter("out", [M, N], f32, isOutput=True)

    with tile.TileContext(nc) as tc:
        with ExitStack() as ctx:
            xs = ctx.enter_context(tc.tile_pool(name="xs", bufs=1))
            xs_f = ctx.enter_context(tc.tile_pool(name="xsf", bufs=2))
            wbt_pool = ctx.enter_context(tc.tile_pool(name="wbt", bufs=4))
            paw = ctx.enter_context(tc.tile_pool(name="paw", bufs=2))
            paa = ctx.enter_context(tc.tile_pool(name="paa", bufs=1))
            paq = ctx.enter_context(tc.tile_pool(name="paq", bufs=1))
            pawb = ctx.enter_context(tc.tile_pool(name="pawb", bufs=3))
            ps_pool = ctx.enter_context(tc.tile_pool(name="ps", bufs=6, space="PSUM"))
            pst_pool = ctx.enter_context(tc.tile_pool(name="pst", bufs=2, space="PSUM"))
            op_pool = ctx.enter_context(tc.tile_pool(name="op", bufs=3))
            cst_pool = ctx.enter_context(tc.tile_pool(name="cst", bufs=1))

            x_r = x_d.rearrange("(ko p) n -> p ko n", p=P)      # (P, KT, N)
            out_r = out_d.rearrange("(mb p) n -> p mb n", p=P)  # (P, MB, N)

            # Identity matrix for PE-mode transposes (lhsT.T @ I).
            iot = cst_pool.tile([P, P], i32)
            nc.gpsimd.iota(iot[:], pattern=[[1, P]], base=0, channel_multiplier=-1)
            ident = cst_pool.tile([P, P], f16)
            nc.vector.tensor_scalar(ident[:], iot[:], 0, None, Alu.is_equal)

            # ---- x load + f16 cast. One full-width row-piece per kt:
            # [P, 1, N] fp32 = 8KB contiguous per partition -> big DMA
            # packets (packet-processing rate, not bytes, limits the DMA
            # fabric here). Loads ride the sync queue; casts alternate
            # scalar/vector so neither engine FIFO paces the stream.
            # Pieces are drip-fed into the emission via emit_x_pieces().
            xh = xs.tile([P, KT, N], f16)
            _x_pieces = list(range(KT))
            _x_pieces.reverse()  # pop from the end

            def emit_x_pieces(n):
                for _ in range(n):
                    if not _x_pieces:
                        return
                    kt = _x_pieces.pop()
                    xf = xs_f.tile([P, 1, N], f32, tag="xf")
                    nc.sync.dma_start(xf[:], x_r[:, kt:kt + 1, :])
                    dst = xh[:, kt:kt + 1, :]
                    nc.scalar.activation(dst, xf[:], Act.Copy)

            # Phase A emitter: one m-block's mask -> lhsT. Phase B emitter:
            # one (mb, nch) PSUM group -> copy -> store. Emission order
            # interleaves them along the diagonal s = mb + nch so the PE
            # queue consumes work in the order masks and x chunks become
            # available (x chunk j lands at ~25*j us; mask i at ~20*i us).
            wbt_tiles = {}

            def phase_a(mb):
                wbt = wbt_pool.tile([P, KT, P], f16, tag="wbt")
                wbt_tiles[mb] = wbt
                for ks in range(KS):
                    wsub = paw.tile([P, k_sub], f32, tag="wsub")
                    nc.gpsimd.dma_start(
                        wsub[:],
                        w_d[mb * P:(mb + 1) * P, ks * k_sub:(ks + 1) * k_sub],
                    )
                    aq = paa.tile([P, k_sub], f32, tag="absq")
                    nc.vector.tensor_scalar(
                        aq[:].bitcast(i32), wsub[:].bitcast(i32),
                        0x7FFFFFFF, None, Alu.bitwise_and,
                    )
                    a4 = aq.rearrange("p (g j) -> p g j", j=4)   # (P, Q, 4)
                    w4 = wsub.rearrange("p (g j) -> p g j", j=4)

                    def tt(name, u, v, op):
                        t_ = paq.tile([P, Q], f32, tag=name)
                        nc.vector.tensor_tensor(t_[:], u, v, op)
                        return t_

                    hi1 = tt("hi1", a4[:, :, 0], a4[:, :, 1], Alu.max)
                    lo1 = tt("lo1", a4[:, :, 0], a4[:, :, 1], Alu.min)
                    hi2 = tt("hi2", a4[:, :, 2], a4[:, :, 3], Alu.max)
                    lo2 = tt("lo2", a4[:, :, 2], a4[:, :, 3], Alu.min)
                    mm = tt("mm", hi1[:], hi2[:], Alu.min)
                    MM = tt("MM", lo1[:], lo2[:], Alu.max)
                    thr = tt("thr", mm[:], MM[:], Alu.max)

                    # wb = (w >= t) over the whole chunk in one contiguous op:
                    # broadcast t across the group axis with a stride-0 AP.
                    wb = pawb.tile([P, k_sub], f16, tag="wbq")
                    wb4 = wb.rearrange("p (g j) -> p g j", j=4)
                    thr3 = thr.rearrange("p (g o) -> p g o", o=1)
                    b0, b1 = broadcast_tensor_aps(w4[:, :, :], thr3[:, :, :])
                    nc.vector.tensor_tensor(wb4[:, :, :], b0, b1, Alu.is_ge)

                    # Transpose wb into lhsT layout on the PE (xbar DMA
                    # transposes bottleneck on single-queue packet rate).
                    # All 8 blocks of the chunk land in one PSUM bank with
                    # a single copy out, so the PE transpose stream only
                    # waits on the scalar engine once per chunk.
                    pt = pst_pool.tile([P, TPS, P], f16, tag="pt")
                    for c in range(TPS):
                        nc.tensor.transpose(
                            pt[:, c, :], wb[:, c * P:(c + 1) * P], ident[:]
                        )
                    nc.scalar.activation(
                        wbt[:, ks * TPS:(ks + 1) * TPS, :], pt[:], Act.Copy
                    )
                    emit_x_pieces(1)

            def phase_b(mb, nch):
                wbt = wbt_tiles[mb]
                ps = ps_pool.tile([P, n_chunk], f32, tag="ps")
                nsl = slice(nch * n_chunk, (nch + 1) * n_chunk)
                for kt in range(KT):
                    nc.tensor.matmul(
                        ps[:],
                        lhsT=wbt[:, kt, :],
                        rhs=xh[:, kt, nsl],
                        start=(kt == 0),
                        stop=(kt == KT - 1),
                    )
                ob = op_pool.tile([P, n_chunk], f32, tag="ob")
                nc.scalar.activation(ob[:], ps[:], Act.Copy)
                nc.scalar.dma_start(
                    out_r[:, mb, nsl], ob[:]
                )
                emit_x_pieces(1)

            # Diagonal emission: work unlocks along s = mb + nch as masks
            # and x pieces become available. phase_a(s+1) is emitted AFTER
            # diagonal s's groups so its PE-transposes (which wait on the
            # DVE) never block diagonal s's matmuls in the tensor FIFO.
            emit_x_pieces(4)
            phase_a(0)
            for s in range(MB + NCH - 1):
                for mb in range(min(s, MB - 1), max(0, s - NCH + 1) - 1, -1):
                    phase_b(mb, s - mb)
                if s + 1 < MB:
                    phase_a(s + 1)
                emit_x_pieces(2)

    nc.finalize()
    return nc


def _get_nc():
    if "nc" not in _CACHE:
        _CACHE["nc"] = _build_bass()
    return _CACHE["nc"]


def make_in_maps(x: np.ndarray, weight: np.ndarray):
    maps = []
    for c in range(N_CORES):
        mi, ni = divmod(c, GRID_N)
        maps.append({
            "w": np.ascontiguousarray(weight[mi * M_SHARD:(mi + 1) * M_SHARD, :]),
            "x": np.ascontiguousarray(x[:, ni * N_SHARD:(ni + 1) * N_SHARD]),
        })
    return maps


def assemble(results) -> np.ndarray:
    out = np.empty((M_FULL, N_FULL), dtype=np.float32)
    for c in range(N_CORES):
        mi, ni = divmod(c, GRID_N)
        out[mi * M_SHARD:(mi + 1) * M_SHARD,
            ni * N_SHARD:(ni + 1) * N_SHARD] = results[c]["out"]
    return out


def kernel(x: np.ndarray, weight: np.ndarray) -> np.ndarray:
    from concourse.bass_utils import run_bass_kernel_spmd

    x = np.ascontiguousarray(np.asarray(x, dtype=np.float32))
    weight = np.ascontiguousarray(np.asarray(weight, dtype=np.float32))
    assert x.shape == (K_FULL, N_FULL) and weight.shape == (M_FULL, K_FULL)

    nc = _get_nc()
    res = run_bass_kernel_spmd(nc, make_in_maps(x, weight), list(range(N_CORES)))
    return assemble(res.results)


# revision 51
# speedup vs baseline: 1.0015x; 1.0015x over previous
"""BinaryLinear 2:4 kernel for trn2 (8 NeuronCores).

Computes: out = binarize(weight * mask_2_4(weight)) @ x
  - mask_2_4: keep 2 largest-|.| of every 4 along the reduction dim
  - binarize: kept positive -> 1.0, else 0.0
  - out = wb @ x, (4096, 4096) fp32

Sharding: 4 (weight rows) x 2 (x cols) grid. Core c=(mi,ni) takes weight
rows [mi*1024,(mi+1)*1024) and x cols [ni*2048,(ni+1)*2048), computing a
(1024, 2048) block; host assembles. Per-core HBM traffic 56MiB (~160us)
stays under the f16 tensor-engine floor (~220us).

Per-core schedule (mb-major wavefront): all four 512-col x chunks stay
resident in SBUF as f16 (128KiB/partition). For each 128-row weight block:
compute the 2:4 binarized mask (phase A), transpose it into a rotating
lhsT buffer, then immediately run all 4 n-chunk PSUM groups (27.6us of PE
work) against it. Mask production (~22us/block) stays ahead of PE
consumption, so the PE streams without long stalls and HAM stays warm.

Phase A per (128, 2048) chunk, all exact fp32 on DVE:
  t = 2nd-largest(|w|) per group of 4 = max(min(hi1,hi2), max(lo1,lo2))
  wb_j = (w_j >= t)  -- one contiguous op with a stride-0 broadcast of t
  (sign folded in since t > 0; verified bit-identical to jax top_k mask)
"""

import numpy as np

M_FULL = 4096
K_FULL = 4096
N_FULL = 4096
N_CORES = 8
GRID_M = 4
GRID_N = 2
M_SHARD = M_FULL // GRID_M   # 1024 weight rows per core
N_SHARD = N_FULL // GRID_N   # 2048 x cols per core

_CACHE = {}


def _build_bass(M=M_SHARD, K=K_FULL, N=N_SHARD, k_sub=1024, n_chunk=512, xp=4):
    import concourse.bass as bass
    import concourse.tile as tile
    from concourse import bacc, mybir
    from concourse.bass import broadcast_tensor_aps
    from contextlib import ExitStack

    dt = mybir.dt
    f32 = dt.float32
    f16 = dt.float16
    i32 = dt.int32
    Alu = mybir.AluOpType
    Act = mybir.ActivationFunctionType

    P = 128
    MB = M // P            # 8 m-blocks per core
    KS = K // k_sub        # 2 phase-A chunks per m-block
    KT = K // P            # 32 contraction tiles
    NCH = N // n_chunk     # 4 x chunks, all resident
    Q = k_sub // 4         # 512 groups per chunk
    TPS = k_sub // P       # 16 k-tiles per chunk

    nc = bacc.Bacc()
    w_d = nc.declare_dram_parameter("w", [M, K], f32, isOutput=False)
    x_d = nc.declare_dram_parameter("x", [K, N], f32, isOutput=False)
    out_d = nc.declare_dram_parameter("out", [M, N], f32, isOutput=True)

    with tile.TileContext(nc) as tc:
        with ExitStack() as ctx:
            xs = ctx.enter_context(tc.tile_pool(name="xs", bufs=1))
            xs_f = ctx.enter_context(tc.tile_pool(name="xsf", bufs=2))
            wbt_pool = ctx.enter_context(tc.tile_pool(name="wbt", bufs=4))
            paw = ctx.enter_context(tc.tile_pool(name="paw", bufs=2))
            paa = ctx.enter_context(tc.tile_pool(name="paa", bufs=1))
            paq = ctx.enter_context(tc.tile_pool(name="paq", bufs=1))
            pawb = ctx.enter_context(tc.tile_pool(name="pawb", bufs=3))
            ps_pool = ctx.enter_context(tc.tile_pool(name="ps", bufs=6, space="PSUM"))
            pst_pool = ctx.enter_context(tc.tile_pool(name="pst", bufs=2, space="PSUM"))
            op_pool = ctx.enter_context(tc.tile_pool(name="op", bufs=3))
            cst_pool = ctx.enter_context(tc.tile_pool(name="cst", bufs=1))

            x_r = x_d.rearrange("(ko p) n -> p ko n", p=P)      # (P, KT, N)
            out_r = out_d.rearrange("(mb p) n -> p mb n", p=P)  # (P, MB, N)

            # Identity matrix for PE-mode transposes (lhsT.T @ I).
            iot = cst_pool.tile([P, P], i32)
            nc.gpsimd.iota(iot[:], pattern=[[1, P]], base=0, channel_multiplier=-1)
            ident = cst_pool.tile([P, P], f16)
            nc.vector.tensor_scalar(ident[:], iot[:], 0, None, Alu.is_equal)

            # PE warm-up: ~50 dummy transposes at the queue head keep the
            # HAM activity window busy from t~3us so the first real
            # transposes and matmul diagonals run at 2.4GHz, not 1.2.
            wup = pst_pool.tile([P, 4, P], f16, tag="pt")
            for i in range(48):
                nc.tensor.transpose(wup[:, i % 4, :], ident[:], ident[:])

            # ---- x load + f16 cast. One full-width row-piece per kt:
            # [P, 1, N] fp32 = 8KB contiguous per partition -> big DMA
            # packets (packet-processing rate, not bytes, limits the DMA
            # fabric here). Loads ride the sync queue; casts alternate
            # scalar/vector so neither engine FIFO paces the stream.
            # Pieces are drip-fed into the emission via emit_x_pieces().
            xh = xs.tile([P, KT, N], f16)
            _x_pieces = list(range(KT))
            _x_pieces.reverse()  # pop from the end

            def emit_x_pieces(n):
                for _ in range(n):
                    if not _x_pieces:
                        return
                    kt = _x_pieces.pop()
                    xf = xs_f.tile([P, 1, N], f32, tag="xf")
                    nc.sync.dma_start(xf[:], x_r[:, kt:kt + 1, :])
                    dst = xh[:, kt:kt + 1, :]
                    nc.scalar.activation(dst, xf[:], Act.Copy)

            # Phase A emitter: one m-block's mask -> lhsT. Phase B emitter:
            # one (mb, nch) PSUM group -> copy -> store. Emission order
            # interleaves them along the diagonal s = mb + nch so the PE
            # queue consumes work in the order masks and x chunks become
            # available (x chunk j lands at ~25*j us; mask i at ~20*i us).
            wbt_tiles = {}

            def phase_a(mb):
                wbt = wbt_pool.tile([P, KT, P], f16, tag="wbt")
                wbt_tiles[mb] = wbt
                for ks in range(KS):
                    wsub = paw.tile([P, k_sub], f32, tag="wsub")
                    nc.gpsimd.dma_start(
                        wsub[:],
                        w_d[mb * P:(mb + 1) * P, ks * k_sub:(ks + 1) * k_sub],
                    )
                    aq = paa.tile([P, k_sub], f32, tag="absq")
                    nc.vector.tensor_scalar(
                        aq[:].bitcast(i32), wsub[:].bitcast(i32),
                        0x7FFFFFFF, None, Alu.bitwise_and,
                    )
                    a4 = aq.rearrange("p (g j) -> p g j", j=4)   # (P, Q, 4)
                    w4 = wsub.rearrange("p (g j) -> p g j", j=4)

                    def tt(name, u, v, op):
                        t_ = paq.tile([P, Q], f32, tag=name)
                        nc.vector.tensor_tensor(t_[:], u, v, op)
                        return t_

                    hi1 = tt("hi1", a4[:, :, 0], a4[:, :, 1], Alu.max)
                    lo1 = tt("lo1", a4[:, :, 0], a4[:, :, 1], Alu.min)
                    hi2 = tt("hi2", a4[:, :, 2], a4[:, :, 3], Alu.max)
                    lo2 = tt("lo2", a4[:, :, 2], a4[:, :, 3], Alu.min)
                    mm = tt("mm", hi1[:], hi2[:], Alu.min)
                    MM = tt("MM", lo1[:], lo2[:], Alu.max)
                    thr = tt("thr", mm[:], MM[:], Alu.max)

                    # wb = (w >= t) over the whole chunk in one contiguous op:
                    # broadcast t across the group axis with a stride-0 AP.
                    wb = pawb.tile([P, k_sub], f16, tag="wbq")
                    wb4 = wb.rearrange("p (g j) -> p g j", j=4)
                    thr3 = thr.rearrange("p (g o) -> p g o", o=1)
                    b0, b1 = broadcast_tensor_aps(w4[:, :, :], thr3[:, :, :])
                    nc.vector.tensor_tensor(wb4[:, :, :], b0, b1, Alu.is_ge)

                    # Transpose wb into lhsT layout on the PE (xbar DMA
                    # transposes bottleneck on single-queue packet rate).
                    for g in range(TPS // 4):
                        pt = pst_pool.tile([P, 4, P], f16, tag="pt")
                        for c4 in range(4):
                            c = g * 4 + c4
                            nc.tensor.transpose(
                                pt[:, c4, :], wb[:, c * P:(c + 1) * P], ident[:]
                            )
                        kt0 = ks * TPS + g * 4
                        nc.scalar.activation(wbt[:, kt0:kt0 + 4, :], pt[:], Act.Copy)
                    emit_x_pieces(1)

            def phase_b(mb, nch):
                wbt = wbt_tiles[mb]
                ps = ps_pool.tile([P, n_chunk], f32, tag="ps")
                nsl = slice(nch * n_chunk, (nch + 1) * n_chunk)
                for kt in range(KT):
                    nc.tensor.matmul(
                        ps[:],
                        lhsT=wbt[:, kt, :],
                        rhs=xh[:, kt, nsl],
                        start=(kt == 0),
                        stop=(kt == KT - 1),
                    )
                ob = op_pool.tile([P, n_chunk], f32, tag="ob")
                nc.scalar.activation(ob[:], ps[:], Act.Copy)
                nc.scalar.dma_start(
                    out_r[:, mb, nsl], ob[:]
                )
                emit_x_pieces(1)

            # Diagonal emission: work unlocks along s = mb + nch as masks
            # and x pieces become available. phase_a(s+1) is emitted AFTER
            # diagonal s's groups so its PE-transposes (which wait on the
            # DVE) never block diagonal s's matmuls in the tensor FIFO.
            emit_x_pieces(4)
            phase_a(0)
            for s in range(MB + NCH - 1):
                for mb in range(min(s, MB - 1), max(0, s - NCH + 1) - 1, -1):
                    phase_b(mb, s - mb)
                if s + 1 < MB:
                    phase_a(s + 1)
                emit_x_pieces(2)

    nc.finalize()
    return nc


def _get_nc():
    if "nc" not in _CACHE:
        _CACHE["nc"] = _build_bass()
    return _CACHE["nc"]


def make_in_maps(x: np.ndarray, weight: np.ndarray):
    maps = []
    for c in range(N_CORES):
        mi, ni = divmod(c, GRID_N)
        maps.append({
            "w": np.ascontiguousarray(weight[mi * M_SHARD:(mi + 1) * M_SHARD, :]),
            "x": np.ascontiguousarray(x[:, ni * N_SHARD:(ni + 1) * N_SHARD]),
        })
    return maps


def assemble(results) -> np.ndarray:
    out = np.empty((M_FULL, N_FULL), dtype=np.float32)
    for c in range(N_CORES):
        mi, ni = divmod(c, GRID_N)
        out[mi * M_SHARD:(mi + 1) * M_SHARD,
            ni * N_SHARD:(ni + 1) * N_SHARD] = results[c]["out"]
    return out


def kernel(x: np.ndarray, weight: np.ndarray) -> np.ndarray:
    from concourse.bass_utils import run_bass_kernel_spmd

    x = np.ascontiguousarray(np.asarray(x, dtype=np.float32))
    weight = np.ascontiguousarray(np.asarray(weight, dtype=np.float32))
    assert x.shape == (K_FULL, N_FULL) and weight.shape == (M_FULL, K_FULL)

    nc = _get_nc()
    res = run_bass_kernel_spmd(nc, make_in_maps(x, weight), list(range(N_CORES)))
    return assemble(res.results)


# revision 53
# speedup vs baseline: 1.0934x; 1.0918x over previous
"""BinaryLinear 2:4 kernel for trn2 (8 NeuronCores).

Computes: out = binarize(weight * mask_2_4(weight)) @ x
  - mask_2_4: keep 2 largest-|.| of every 4 along the reduction dim
  - binarize: kept positive -> 1.0, else 0.0
  - out = wb @ x, (4096, 4096) fp32

Sharding: 4 (weight rows) x 2 (x cols) grid. Core c=(mi,ni) takes weight
rows [mi*1024,(mi+1)*1024) and x cols [ni*2048,(ni+1)*2048), computing a
(1024, 2048) block; host assembles. Per-core HBM traffic 56MiB (~160us)
stays under the f16 tensor-engine floor (~220us).

Per-core schedule (mb-major wavefront): all four 512-col x chunks stay
resident in SBUF as f16 (128KiB/partition). For each 128-row weight block:
compute the 2:4 binarized mask (phase A), transpose it into a rotating
lhsT buffer, then immediately run all 4 n-chunk PSUM groups (27.6us of PE
work) against it. Mask production (~22us/block) stays ahead of PE
consumption, so the PE streams without long stalls and HAM stays warm.

Phase A per (128, 2048) chunk, all exact fp32 on DVE:
  t = 2nd-largest(|w|) per group of 4 = max(min(hi1,hi2), max(lo1,lo2))
  wb_j = (w_j >= t)  -- one contiguous op with a stride-0 broadcast of t
  (sign folded in since t > 0; verified bit-identical to jax top_k mask)
"""

import numpy as np

M_FULL = 4096
K_FULL = 4096
N_FULL = 4096
N_CORES = 8
GRID_M = 4
GRID_N = 2
M_SHARD = M_FULL // GRID_M   # 1024 weight rows per core
N_SHARD = N_FULL // GRID_N   # 2048 x cols per core

_CACHE = {}


def _build_bass(M=M_SHARD, K=K_FULL, N=N_SHARD, k_sub=1024, n_chunk=512, xp=4):
    import concourse.bass as bass
    import concourse.tile as tile
    from concourse import bacc, mybir
    from concourse.bass import broadcast_tensor_aps
    from contextlib import ExitStack

    dt = mybir.dt
    f32 = dt.float32
    f16 = dt.float16
    i32 = dt.int32
    Alu = mybir.AluOpType
    Act = mybir.ActivationFunctionType

    P = 128
    MB = M // P            # 8 m-blocks per core
    KS = K // k_sub        # 2 phase-A chunks per m-block
    KT = K // P            # 32 contraction tiles
    NCH = N // n_chunk     # 4 x chunks, all resident
    Q = k_sub // 4         # 512 groups per chunk
    TPS = k_sub // P       # 16 k-tiles per chunk

    nc = bacc.Bacc()
    w_d = nc.declare_dram_parameter("w", [M, K], f32, isOutput=False)
    x_d = nc.declare_dram_parameter("x", [K, N], f32, isOutput=False)
    out_d = nc.declare_dram_parameter("out", [M, N], f32, isOutput=True)

    with tile.TileContext(nc) as tc:
        with ExitStack() as ctx:
            xs = ctx.enter_context(tc.tile_pool(name="xs", bufs=1))
            xs_f = ctx.enter_context(tc.tile_pool(name="xsf", bufs=2))
            wbt_pool = ctx.enter_context(tc.tile_pool(name="wbt", bufs=4))
            paw = ctx.enter_context(tc.tile_pool(name="paw", bufs=2))
            paa = ctx.enter_context(tc.tile_pool(name="paa", bufs=1))
            paq = ctx.enter_context(tc.tile_pool(name="paq", bufs=1))
            pawb = ctx.enter_context(tc.tile_pool(name="pawb", bufs=3))
            ps_pool = ctx.enter_context(tc.tile_pool(name="ps", bufs=6, space="PSUM"))
            pst_pool = ctx.enter_context(tc.tile_pool(name="pst", bufs=2, space="PSUM"))
            op_pool = ctx.enter_context(tc.tile_pool(name="op", bufs=3))
            cst_pool = ctx.enter_context(tc.tile_pool(name="cst", bufs=1))

            x_r = x_d.rearrange("(ko p) n -> p ko n", p=P)      # (P, KT, N)
            out_r = out_d.rearrange("(mb p) n -> p mb n", p=P)  # (P, MB, N)

            # Identity matrix for PE-mode transposes (lhsT.T @ I).
            iot = cst_pool.tile([P, P], i32)
            nc.gpsimd.iota(iot[:], pattern=[[1, P]], base=0, channel_multiplier=-1)
            ident = cst_pool.tile([P, P], f16)
            nc.vector.tensor_scalar(ident[:], iot[:], 0, None, Alu.is_equal)

            # ---- x load + f16 cast. One full-width row-piece per kt:
            # [P, 1, N] fp32 = 8KB contiguous per partition -> big DMA
            # packets (packet-processing rate, not bytes, limits the DMA
            # fabric here). Loads ride the sync queue; casts alternate
            # scalar/vector so neither engine FIFO paces the stream.
            # Pieces are drip-fed into the emission via emit_x_pieces().
            xh = xs.tile([P, KT, N], f16)
            _x_pieces = list(range(KT))
            _x_pieces.reverse()  # pop from the end

            def emit_x_pieces(n):
                for _ in range(n):
                    if not _x_pieces:
                        return
                    kt = _x_pieces.pop()
                    xf = xs_f.tile([P, 1, N], f32, tag="xf")
                    nc.sync.dma_start(xf[:], x_r[:, kt:kt + 1, :])
                    dst = xh[:, kt:kt + 1, :]
                    nc.scalar.activation(dst, xf[:], Act.Copy)

            # Phase A emitter: one m-block's mask -> lhsT. Phase B emitter:
            # one (mb, nch) PSUM group -> copy -> store. Emission order
            # interleaves them along the diagonal s = mb + nch so the PE
            # queue consumes work in the order masks and x chunks become
            # available (x chunk j lands at ~25*j us; mask i at ~20*i us).
            wbt_tiles = {}

            def phase_a(mb):
                wbt = wbt_pool.tile([P, KT, P], f16, tag="wbt")
                wbt_tiles[mb] = wbt
                for ks in range(KS):
                    wsub = paw.tile([P, k_sub], f32, tag="wsub")
                    nc.gpsimd.dma_start(
                        wsub[:],
                        w_d[mb * P:(mb + 1) * P, ks * k_sub:(ks + 1) * k_sub],
                    )
                    aq = paa.tile([P, k_sub], f32, tag="absq")
                    nc.vector.tensor_scalar(
                        aq[:].bitcast(i32), wsub[:].bitcast(i32),
                        0x7FFFFFFF, None, Alu.bitwise_and,
                    )
                    a4 = aq.rearrange("p (g j) -> p g j", j=4)   # (P, Q, 4)
                    w4 = wsub.rearrange("p (g j) -> p g j", j=4)

                    def tt(name, u, v, op):
                        t_ = paq.tile([P, Q], f32, tag=name)
                        nc.vector.tensor_tensor(t_[:], u, v, op)
                        return t_

                    hi1 = tt("hi1", a4[:, :, 0], a4[:, :, 1], Alu.max)
                    lo1 = tt("lo1", a4[:, :, 0], a4[:, :, 1], Alu.min)
                    hi2 = tt("hi2", a4[:, :, 2], a4[:, :, 3], Alu.max)
                    lo2 = tt("lo2", a4[:, :, 2], a4[:, :, 3], Alu.min)
                    mm = tt("mm", hi1[:], hi2[:], Alu.min)
                    MM = tt("MM", lo1[:], lo2[:], Alu.max)
                    thr = tt("thr", mm[:], MM[:], Alu.max)

                    # wb = (w >= t) over the whole chunk in one contiguous op:
                    # broadcast t across the group axis with a stride-0 AP.
                    wb = pawb.tile([P, k_sub], f16, tag="wbq")
                    wb4 = wb.rearrange("p (g j) -> p g j", j=4)
                    thr3 = thr.rearrange("p (g o) -> p g o", o=1)
                    b0, b1 = broadcast_tensor_aps(w4[:, :, :], thr3[:, :, :])
                    nc.vector.tensor_tensor(wb4[:, :, :], b0, b1, Alu.is_ge)

                    # Transpose wb into lhsT layout on the PE (xbar DMA
                    # transposes bottleneck on single-queue packet rate).
                    for g in range(TPS // 4):
                        pt = pst_pool.tile([P, 4, P], f16, tag="pt")
                        for c4 in range(4):
                            c = g * 4 + c4
                            nc.tensor.transpose(
                                pt[:, c4, :], wb[:, c * P:(c + 1) * P], ident[:]
                            )
                        kt0 = ks * TPS + g * 4
                        nc.scalar.activation(wbt[:, kt0:kt0 + 4, :], pt[:], Act.Copy)
                    emit_x_pieces(1)

            def phase_b(mb, nch):
                wbt = wbt_tiles[mb]
                ps = ps_pool.tile([P, n_chunk], f32, tag="ps")
                nsl = slice(nch * n_chunk, (nch + 1) * n_chunk)
                for kt in range(KT):
                    nc.tensor.matmul(
                        ps[:],
                        lhsT=wbt[:, kt, :],
                        rhs=xh[:, kt, nsl],
                        start=(kt == 0),
                        stop=(kt == KT - 1),
                    )
                ob = op_pool.tile([P, n_chunk], f32, tag="ob")
                nc.scalar.activation(ob[:], ps[:], Act.Copy)
                nc.scalar.dma_start(
                    out_r[:, mb, nsl], ob[:]
                )
                emit_x_pieces(1)

            # Diagonal emission: work unlocks along s = mb + nch as masks
            # and x pieces become available. phase_a(s+1) is emitted AFTER
            # diagonal s's groups so its PE-transposes (which wait on the
            # DVE) never block diagonal s's matmuls in the tensor FIFO.
            emit_x_pieces(4)
            phase_a(0)
            for s in range(MB + NCH - 1):
                # Oldest-mask groups first: the newest mask's group runs
                # last, giving its DVE+transpose pipeline ~20us of runway.
                for mb in range(max(0, s - NCH + 1), min(s, MB - 1) + 1):
                    phase_b(mb, s - mb)
                if s + 1 < MB:
                    phase_a(s + 1)
                emit_x_pieces(2)

    nc.finalize()
    return nc


def _get_nc():
    if "nc" not in _CACHE:
        _CACHE["nc"] = _build_bass()
    return _CACHE["nc"]


def make_in_maps(x: np.ndarray, weight: np.ndarray):
    maps = []
    for c in range(N_CORES):
        mi, ni = divmod(c, GRID_N)
        maps.append({
            "w": np.ascontiguousarray(weight[mi * M_SHARD:(mi + 1) * M_SHARD, :]),
            "x": np.ascontiguousarray(x[:, ni * N_SHARD:(ni + 1) * N_SHARD]),
        })
    return maps


def assemble(results) -> np.ndarray:
    out = np.empty((M_FULL, N_FULL), dtype=np.float32)
    for c in range(N_CORES):
        mi, ni = divmod(c, GRID_N)
        out[mi * M_SHARD:(mi + 1) * M_SHARD,
            ni * N_SHARD:(ni + 1) * N_SHARD] = results[c]["out"]
    return out


def kernel(x: np.ndarray, weight: np.ndarray) -> np.ndarray:
    from concourse.bass_utils import run_bass_kernel_spmd

    x = np.ascontiguousarray(np.asarray(x, dtype=np.float32))
    weight = np.ascontiguousarray(np.asarray(weight, dtype=np.float32))
    assert x.shape == (K_FULL, N_FULL) and weight.shape == (M_FULL, K_FULL)

    nc = _get_nc()
    res = run_bass_kernel_spmd(nc, make_in_maps(x, weight), list(range(N_CORES)))
    return assemble(res.results)
